# revision 22
# baseline (speedup 1.0000x reference)
"""Trainium2 Bass kernel for nn_Neighbor_Mean (gnn message passing).

Math: out[b,s,:] = mean_n( mask[b,s,n] * (T_b[idx[b,s,n]] @ Wn^T) )
 with T_b[v] = pos_table[v] + (h[b][v-1] if v>=1 else 0)   (v in [0, 2049))
Since the mask multiplies matmul outputs and matmul is linear:
 out[b,s,:] = ( (1/N) * sum_n T'_b[idx_eff[b,s,n]] ) @ Wn^T
 where T' has an extra zero row at SINK=S+1 and idx_eff = mask ? idx : SINK.

Sharding: data-parallel over batch, one NeuronCore per batch row (B == 8).

Per-core plan (v6):
 - table T' in SBUF as bf16, slot g(v) = 128*(v%17) + v//17 (partition
   p = v//17, rank r = v%17). The host pads pos/new_h with zero rows to
   17*128 rows, so each staging load is ONE contiguous-per-partition DMA
   (128 descriptors of 8.5KB); host remaps indices to g(idx) and passes
   nh = concat([zeros, h]). The sink slot is zero via the padding.
 - indices/mask loaded contiguously into [16, S*N/16] int32 (partition =
   s//128, col = (s%128)*N + n). ONE DVE copy_predicated on int16 views
   does masked-select + int32->int16 + the (k w n)->(k n w) permutation
   in a single pass into a sink-prefilled idxbuf; replicate to the 8
   16-partition groups by doubling.
 - gpsimd.load_library(mlp) issued first so the Q7 ext-isa reload
   (~12us) overlaps the prologue instead of stalling the first gather.
 - SBUF->SBUF transposed dma_gather, 512 idx/call (ucode ring ceiling),
   4 SWDGE queues; desc-gen on GpSimd is the serial bottleneck
   (~2.9us/call, ~370us total).
 - pipeline at 4096-position chunk granularity (8 calls + 32 matmuls per
   chunk, 4 chunk tiles in flight): psum row m = 16w + p -> out row
   s = 128p + 8k + w via a strided DRAM AP on the output DMA.

IMPORTANT: all 2-read DVE ops (copy_predicated, tensor_add) must finish
before any dma_gather runs -- the gather ucode streams its indices
through the POOL/DVE shared SBUF read port, and a concurrent 2-port DVE
op corrupts the stream. All gathers transitively depend on the DVE
prologue via tbl/idxbuf.
"""
import sys

sys.path.insert(0, '/opt/trn_rl_repo')

import numpy as np

import concourse.bacc as bacc
import concourse.bass as bass
import concourse.mybir as mybir
import concourse.tile as tile
from concourse import library_config
from concourse.bass_utils import run_bass_kernel_spmd
from concourse.masks import make_identity

B, N, H = 8, 32, 128
NI = 512             # idxs per dma_gather call (ucode ring ceiling)
CHUNK = 4096         # gather positions per pipeline chunk (= 128 s rows)
F32 = mybir.dt.float32
I32 = mybir.dt.int32
I16 = mybir.dt.int16
BF16 = mybir.dt.bfloat16


def build_program(S: int = 2048, ni: int = NI):
    VPOS = S + 1                      # pos_table rows; sink index == VPOS
    NRANKS = (VPOS + 1 + 127) // 128  # table ranks (17 for S=2048)
    nchunk = S * N // CHUNK
    calls = CHUNK // ni               # gather calls per chunk
    assert S % 128 == 0 and CHUNK % ni == 0
    assert ni <= 512, "dma_gather ucode ring ceiling is 512 idxs/call"

    nc = bacc.Bacc("TRN2", debug=False, num_swdge_queues=4)
    # nh = reference's new_h = concat([zeros(1,H), h]) padded with zero rows
    # to NRANKS*128 rows; pos likewise zero-padded.
    nh_d = nc.dram_tensor("nh", [NRANKS * 128, H], F32, kind="ExternalInput")
    idx_d = nc.dram_tensor("idx", [S, N], I32, kind="ExternalInput")
    msk_d = nc.dram_tensor("msk", [S, N], I32, kind="ExternalInput")
    pos_d = nc.dram_tensor("pos", [NRANKS * 128, H], F32, kind="ExternalInput")
    wn_d = nc.dram_tensor("wn", [H, H], F32, kind="ExternalInput")
    out_d = nc.dram_tensor("out", [S, H], F32, kind="ExternalOutput")

    with tile.TileContext(nc) as tc:
        with (
            tc.tile_pool(name="const", bufs=1) as constp,
            tc.tile_pool(name="stage", bufs=1) as stagep,
            tc.tile_pool(name="idxp", bufs=1) as idxp,
            tc.tile_pool(name="gbig", bufs=4) as gbigp,
            tc.tile_pool(name="outp", bufs=4) as outp,
            tc.tile_pool(name="psum", bufs=4, space="PSUM") as psump,
        ):
            # preload the Q7 ext-isa library holding DMAGatherAnt so the
            # ~12us IRAM reload overlaps the rest of the prologue
            nc.gpsimd.load_library(library_config.mlp)

            acols = S * N // 16
            # ---- index/mask loads (head of the sync ring) ------------
            idxw32 = idxp.tile([16, acols], I32, tag="idxw32")
            mskw32 = idxp.tile([16, acols], I32, tag="mskw32")
            nc.sync.dma_start(
                idxw32[:], idx_d[:].rearrange("(p r) n -> p (r n)", p=16)
            )
            nc.sync.dma_start(
                mskw32[:], msk_d[:].rearrange("(p r) n -> p (r n)", p=16)
            )

            # ---- table staging loads ---------------------------------
            # slot g(v) = 128*(v%17) + v//17: tbl[p, r*H:(r+1)*H] =
            # T'[17p + r]; one contiguous descriptor per partition.
            pstage = stagep.tile([128, NRANKS * H], F32, tag="pstage")
            hstage = stagep.tile([128, NRANKS * H], F32, tag="hstage")
            nc.sync.dma_start(
                pstage[:], pos_d[:].rearrange("(p r) e -> p (r e)", p=128)
            )
            nc.scalar.dma_start(
                hstage[:], nh_d[:].rearrange("(p r) e -> p (r e)", p=128)
            )
            wn_sb = constp.tile([H, H], F32)
            nc.scalar.dma_start(wn_sb[:], wn_d[:])

            # ---- fused masked-select + int16 convert + permutation ---
            # The sink lives at table slot 0 (host rolls the padded tables
            # by one row and remaps idx to sigma(idx+1)), so the masked
            # select is just idx_eff = mask * idx_g. ONE strided tensor_mul
            # does select + int32->int16 + the (k w n) -> (k n w)
            # permutation: position col = 256k + 8n + w reads
            # idx[s = 128p + 8k + w, n].
            idxbuf = idxp.tile([128, acols], I16, tag="idxbuf")
            dst = idxbuf[0:16, :].rearrange(
                "p (k n w one) -> p k n w one", n=N, w=8, one=1
            )
            data = idxw32[:].rearrange(
                "p (k w n one) -> p k n w one", w=8, n=N, one=1
            )
            mask = mskw32[:].rearrange(
                "p (k w n one) -> p k n w one", w=8, n=N, one=1
            )
            nc.vector.tensor_mul(dst, data, mask)
            # replicate to the other 7 16-partition groups (independent
            # DMAs split over both HWDGE rings)
            for r in range(1, 8):
                eng = nc.sync if r % 2 else nc.scalar
                eng.dma_start(idxbuf[16 * r:16 * (r + 1), :], idxbuf[0:16, :])

            # ---- table add (DVE) + Wn^T * (1/N) ----------------------
            tbl = constp.tile([128, NRANKS * H], BF16)
            nc.vector.tensor_add(tbl[:], pstage[:], hstage[:])
            ident = constp.tile([128, 128], F32)
            make_identity(nc, ident[:])
            wnt_ps = psump.tile([128, H], F32)
            nc.tensor.transpose(out=wnt_ps[:], in_=wn_sb[:], identity=ident[:])
            wnt = constp.tile([H, H], BF16)
            nc.scalar.mul(wnt[:], wnt_ps[:], 1.0 / N)

            for kg in range(nchunk):
                # ---- gathers: chunk kg covers positions [4096kg, ...) -
                gb = gbigp.tile([128, 1, CHUNK], BF16, tag="gb")
                for c in range(calls):
                    ci = kg * calls + c
                    nc.gpsimd.dma_gather(
                        gb[:, :, c * ni:(c + 1) * ni],
                        tbl[:],
                        idxbuf[:, ci * (ni // 16):(ci + 1) * (ni // 16)],
                        ni, ni, H,
                        transpose=True,
                        queue_num=ci % 4,
                        sbuf_tokens_per_rank=128,
                        sbuf_free_dim_per_rank=H * 2,
                    )

                # ---- matmuls: psum[m,k] += g[h, cols]^T @ wnt --------
                # column j = 128n + 16w + p holds (s = 128p + 8kg + w, n)
                gv = gb[:, 0, :]
                ps = psump.tile([128, H], F32, tag="ps")
                for n in range(N):
                    nc.tensor.matmul(
                        out=ps[:],
                        lhsT=gv[:, 128 * n:128 * n + 128],
                        rhs=wnt[:],
                        start=(n == 0),
                        stop=(n == N - 1),
                    )
                osb = outp.tile([128, H], F32, tag="osb")
                nc.vector.tensor_copy(osb[:], ps[:])
                # psum row m = 16w + p -> out row s = 128p + 8kg + w
                dst2 = out_d[:].rearrange("(p r) e -> p r e", p=16)[
                    :, 8 * kg:8 * kg + 8, :
                ].rearrange("p w e -> w p e")
                nc.sync.dma_start(dst2, osb[:])

    nc.compile()
    return nc


_CACHE: dict[tuple, object] = {}


def _get_program(S: int, ni: int = NI):
    key = (S, ni)
    if key not in _CACHE:
        _CACHE[key] = build_program(S, ni)
    return _CACHE[key]


def _prep_host(h, idx, pos, s):
    """Host-side layout prep: padded tables rolled by one row (DRAM row 0 =
    the sink zero row, row 1+v = T'-source for v) and indices remapped to
    slots sigma(idx+1), sigma(rho) = 128*(rho%17) + rho//17 -- so a masked
    index is just slot 0 (see build_program docstring)."""
    nranks = (s + 2 + 127) // 128
    rows = nranks * 128
    nh = np.zeros((B, rows, H), dtype=np.float32)
    nh[:, 2:s + 2] = h          # row 1+v holds new_h[v] = h[v-1]; rows 0,1 zero
    pos_pad = np.zeros((rows, H), dtype=np.float32)
    pos_pad[1:s + 2] = pos      # row 1+v holds pos[v]
    rho = idx.astype(np.int64) + 1
    idx_g = (128 * (rho % nranks) + rho // nranks).astype(np.int32)
    return nh, pos_pad, idx_g


def kernel(x, h, g, neighbor_index, neighbor_mask, pos_table, Wn):
    """Full inputs in, full output out. x and g are unused by the math
    (g only provides the zero row shape; x is unused in the reference)."""
    h = np.asarray(h, dtype=np.float32)
    idx = np.asarray(neighbor_index)
    msk = np.asarray(neighbor_mask)
    pos = np.asarray(pos_table, dtype=np.float32)
    wn = np.ascontiguousarray(np.asarray(Wn), dtype=np.float32)
    b, s, n = idx.shape
    assert (b, n) == (B, N) and h.shape == (B, s, H)

    nc = _get_program(s)
    nh, pos_pad, idx_g = _prep_host(h, idx, pos, s)
    in_maps = [
        {
            "nh": np.ascontiguousarray(nh[c]),
            "idx": np.ascontiguousarray(idx_g[c]),
            "msk": np.ascontiguousarray(msk[c], dtype=np.int32),
            "pos": pos_pad,
            "wn": wn,
        }
        for c in range(B)
    ]
    res = run_bass_kernel_spmd(nc, in_maps, core_ids=list(range(B)))
    return np.stack([res.results[c]["out"] for c in range(B)], axis=0)


# revision 25
# speedup vs baseline: 1.0976x; 1.0976x over previous
"""Trainium2 Bass kernel for nn_Neighbor_Mean (gnn message passing).

Math: out[b,s,:] = mean_n( mask[b,s,n] * (T_b[idx[b,s,n]] @ Wn^T) )
 with T_b[v] = pos_table[v] + (h[b][v-1] if v>=1 else 0)   (v in [0, 2049))
Since the mask multiplies matmul outputs and matmul is linear:
 out[b,s,:] = ( (1/N) * sum_n T'_b[idx_eff[b,s,n]] ) @ Wn^T
 where T' has an extra zero row at SINK=S+1 and idx_eff = mask ? idx : SINK.

Sharding: data-parallel over batch, one NeuronCore per batch row (B == 8).

Per-core plan (v6):
 - table T' in SBUF as bf16, slot g(v) = 128*(v%17) + v//17 (partition
   p = v//17, rank r = v%17). The host pads pos/new_h with zero rows to
   17*128 rows, so each staging load is ONE contiguous-per-partition DMA
   (128 descriptors of 8.5KB); host remaps indices to g(idx) and passes
   nh = concat([zeros, h]). The sink slot is zero via the padding.
 - indices/mask loaded contiguously into [16, S*N/16] int32 (partition =
   s//128, col = (s%128)*N + n). ONE DVE copy_predicated on int16 views
   does masked-select + int32->int16 + the (k w n)->(k n w) permutation
   in a single pass into a sink-prefilled idxbuf; replicate to the 8
   16-partition groups by doubling.
 - gpsimd.load_library(mlp) issued first so the Q7 ext-isa reload
   (~12us) overlaps the prologue instead of stalling the first gather.
 - SBUF->SBUF transposed dma_gather, 512 idx/call (ucode ring ceiling),
   4 SWDGE queues; desc-gen on GpSimd is the serial bottleneck
   (~2.9us/call, ~370us total).
 - pipeline at 4096-position chunk granularity (8 calls + 32 matmuls per
   chunk, 4 chunk tiles in flight): psum row m = 16w + p -> out row
   s = 128p + 8k + w via a strided DRAM AP on the output DMA.

IMPORTANT: all 2-read DVE ops (copy_predicated, tensor_add) must finish
before any dma_gather runs -- the gather ucode streams its indices
through the POOL/DVE shared SBUF read port, and a concurrent 2-port DVE
op corrupts the stream. All gathers transitively depend on the DVE
prologue via tbl/idxbuf.
"""
import sys

sys.path.insert(0, '/opt/trn_rl_repo')

import numpy as np

import concourse.bacc as bacc
import concourse.bass as bass
import concourse.mybir as mybir
import concourse.tile as tile
from concourse import library_config
from concourse.bass_utils import run_bass_kernel_spmd
from concourse.masks import make_identity

B, N, H = 8, 32, 128
NI = 512             # idxs per dma_gather call (ucode ring ceiling)
CHUNK = 4096         # gather positions per pipeline chunk (= 128 s rows)
F32 = mybir.dt.float32
I32 = mybir.dt.int32
I16 = mybir.dt.int16
BF16 = mybir.dt.bfloat16


def build_program(S: int = 2048, ni: int = NI):
    VPOS = S + 1                      # pos_table rows; sink index == VPOS
    NRANKS = (VPOS + 1 + 127) // 128  # table ranks (17 for S=2048)
    nchunk = S * N // CHUNK
    calls = CHUNK // ni               # gather calls per chunk
    assert S % 128 == 0 and CHUNK % ni == 0
    assert ni <= 512, "dma_gather ucode ring ceiling is 512 idxs/call"

    nc = bacc.Bacc("TRN2", debug=False, num_swdge_queues=4)
    # nh = reference's new_h = concat([zeros(1,H), h]) padded with zero rows
    # to NRANKS*128 rows; pos likewise zero-padded.
    nh_d = nc.dram_tensor("nh", [NRANKS * 128, H], F32, kind="ExternalInput")
    idx_d = nc.dram_tensor("idx", [S, N], I32, kind="ExternalInput")
    msk_d = nc.dram_tensor("msk", [S, N], I32, kind="ExternalInput")
    pos_d = nc.dram_tensor("pos", [NRANKS * 128, H], F32, kind="ExternalInput")
    wn_d = nc.dram_tensor("wn", [H, H], F32, kind="ExternalInput")
    out_d = nc.dram_tensor("out", [S, H], F32, kind="ExternalOutput")

    with tile.TileContext(nc) as tc:
        with (
            tc.tile_pool(name="const", bufs=1) as constp,
            tc.tile_pool(name="stage", bufs=1) as stagep,
            tc.tile_pool(name="idxp", bufs=1) as idxp,
            tc.tile_pool(name="gbig", bufs=4) as gbigp,
            tc.tile_pool(name="outp", bufs=4) as outp,
            tc.tile_pool(name="psum", bufs=4, space="PSUM") as psump,
        ):
            # preload the Q7 ext-isa library holding DMAGatherAnt so the
            # ~12us IRAM reload overlaps the rest of the prologue
            nc.gpsimd.load_library(library_config.mlp)

            acols = S * N // 16
            # ---- index/mask loads (head of BOTH rings) ---------------
            idxw32 = idxp.tile([16, acols], I32, tag="idxw32")
            mskw32 = idxp.tile([16, acols], I32, tag="mskw32")
            nc.sync.dma_start(
                idxw32[:], idx_d[:].rearrange("(p r) n -> p (r n)", p=16)
            )
            nc.scalar.dma_start(
                mskw32[:], msk_d[:].rearrange("(p r) n -> p (r n)", p=16)
            )

            # ---- table staging loads ---------------------------------
            # slot g(v) = 128*(v%17) + v//17: tbl[p, r*H:(r+1)*H] =
            # T'[17p + r]; one contiguous descriptor per partition.
            pstage = stagep.tile([128, NRANKS * H], F32, tag="pstage")
            hstage = stagep.tile([128, NRANKS * H], F32, tag="hstage")
            nc.sync.dma_start(
                pstage[:], pos_d[:].rearrange("(p r) e -> p (r e)", p=128)
            )
            nc.scalar.dma_start(
                hstage[:], nh_d[:].rearrange("(p r) e -> p (r e)", p=128)
            )
            wn_sb = constp.tile([H, H], F32)
            nc.scalar.dma_start(wn_sb[:], wn_d[:])

            # ---- fused masked-select + int16 convert + permutation ---
            # idxbuf[0:16] prefilled with the sink slot; ONE copy_predicated
            # on int16 views then writes g(idx) where mask!=0, applying the
            # (k w n) -> (k n w) permutation: position col = 256k + 8n + w
            # reads idx[s = 128p + 8k + w, n]. NOTE: the sink must NOT live
            # on partitions 0-31 (SWDGE descriptor-ring AXI ports) -- the
            # masked half of all table reads hammers the sink's partition.
            idxbuf = idxp.tile([128, acols], I16, tag="idxbuf")
            sink_g = 128 * (VPOS % NRANKS) + VPOS // NRANKS
            nc.vector.memset(idxbuf[0:16, :], sink_g)
            dst = idxbuf[0:16, :].rearrange(
                "p (k n w one) -> p k n w one", n=N, w=8, one=1
            )
            data = idxw32[:].bitcast(I16).rearrange(
                "p (k w n two) -> p k n w two", w=8, n=N, two=2
            )
            mask = mskw32[:].bitcast(I16).rearrange(
                "p (k w n two) -> p k n w two", w=8, n=N, two=2
            )
            nc.vector.copy_predicated(
                dst, mask[:, :, :, :, 0:1], data[:, :, :, :, 0:1]
            )
            # replicate to the 8 16-partition groups by doubling
            nc.sync.dma_start(idxbuf[16:32, :], idxbuf[0:16, :])
            nc.sync.dma_start(idxbuf[32:64, :], idxbuf[0:32, :])
            nc.sync.dma_start(idxbuf[64:128, :], idxbuf[0:64, :])

            # ---- table add (DVE) + Wn^T * (1/N) ----------------------
            tbl = constp.tile([128, NRANKS * H], BF16)
            nc.vector.tensor_add(tbl[:], pstage[:], hstage[:])
            ident = constp.tile([128, 128], F32)
            make_identity(nc, ident[:])
            wnt_ps = psump.tile([128, H], F32)
            nc.tensor.transpose(out=wnt_ps[:], in_=wn_sb[:], identity=ident[:])
            wnt = constp.tile([H, H], BF16)
            nc.scalar.mul(wnt[:], wnt_ps[:], 1.0 / N)

            for kg in range(nchunk):
                # ---- gathers: chunk kg covers positions [4096kg, ...) -
                gb = gbigp.tile([128, 1, CHUNK], BF16, tag="gb")
                for c in range(calls):
                    ci = kg * calls + c
                    nc.gpsimd.dma_gather(
                        gb[:, :, c * ni:(c + 1) * ni],
                        tbl[:],
                        idxbuf[:, ci * (ni // 16):(ci + 1) * (ni // 16)],
                        ni, ni, H,
                        transpose=True,
                        queue_num=ci % 4,
                        sbuf_tokens_per_rank=128,
                        sbuf_free_dim_per_rank=H * 2,
                    )

                # ---- matmuls: psum[m,k] += g[h, cols]^T @ wnt --------
                # column j = 128n + 16w + p holds (s = 128p + 8kg + w, n)
                gv = gb[:, 0, :]
                ps = psump.tile([128, H], F32, tag="ps")
                for n in range(N):
                    nc.tensor.matmul(
                        out=ps[:],
                        lhsT=gv[:, 128 * n:128 * n + 128],
                        rhs=wnt[:],
                        start=(n == 0),
                        stop=(n == N - 1),
                    )
                osb = outp.tile([128, H], F32, tag="osb")
                nc.vector.tensor_copy(osb[:], ps[:])
                # psum row m = 16w + p -> out row s = 128p + 8kg + w
                dst2 = out_d[:].rearrange("(p r) e -> p r e", p=16)[
                    :, 8 * kg:8 * kg + 8, :
                ].rearrange("p w e -> w p e")
                nc.sync.dma_start(dst2, osb[:])

    nc.compile()
    return nc


_CACHE: dict[tuple, object] = {}


def _get_program(S: int, ni: int = NI):
    key = (S, ni)
    if key not in _CACHE:
        _CACHE[key] = build_program(S, ni)
    return _CACHE[key]


def _prep_host(h, idx, pos, s):
    """Host-side layout prep: zero-padded new_h/pos tables and indices
    remapped to slots g(v) = 128*(v%17) + v//17 (see build_program)."""
    nranks = (s + 2 + 127) // 128
    rows = nranks * 128
    nh = np.zeros((B, rows, H), dtype=np.float32)
    nh[:, 1:s + 1] = h          # row v holds new_h[v] = h[v-1]; row 0 zero
    pos_pad = np.zeros((rows, H), dtype=np.float32)
    pos_pad[:s + 1] = pos
    idx_g = (128 * (idx % nranks) + idx // nranks).astype(np.int32)
    return nh, pos_pad, idx_g


def kernel(x, h, g, neighbor_index, neighbor_mask, pos_table, Wn):
    """Full inputs in, full output out. x and g are unused by the math
    (g only provides the zero row shape; x is unused in the reference)."""
    h = np.asarray(h, dtype=np.float32)
    idx = np.asarray(neighbor_index)
    msk = np.asarray(neighbor_mask)
    pos = np.asarray(pos_table, dtype=np.float32)
    wn = np.ascontiguousarray(np.asarray(Wn), dtype=np.float32)
    b, s, n = idx.shape
    assert (b, n) == (B, N) and h.shape == (B, s, H)

    nc = _get_program(s)
    nh, pos_pad, idx_g = _prep_host(h, idx, pos, s)
    in_maps = [
        {
            "nh": np.ascontiguousarray(nh[c]),
            "idx": np.ascontiguousarray(idx_g[c]),
            "msk": np.ascontiguousarray(msk[c], dtype=np.int32),
            "pos": pos_pad,
            "wn": wn,
        }
        for c in range(B)
    ]
    res = run_bass_kernel_spmd(nc, in_maps, core_ids=list(range(B)))
    return np.stack([res.results[c]["out"] for c in range(B)], axis=0)


# revision 31
# speedup vs baseline: 1.4992x; 1.3659x over previous
"""Trainium2 Bass kernel for nn_Neighbor_Mean (gnn message passing).

Math: out[b,s,:] = mean_n( mask[b,s,n] * (T_b[idx[b,s,n]] @ Wn^T) )
 with T_b[v] = pos_table[v] + (h[b][v-1] if v>=1 else 0)   (v in [0, 2049))
Since the mask multiplies matmul outputs and matmul is linear:
 out[b,s,:] = ( (1/N) * sum_n T'_b[idx_eff[b,s,n]] ) @ Wn^T
 where T' has an extra zero row at SINK=S+1 and idx_eff = mask ? idx : SINK.

Sharding: data-parallel over batch, one NeuronCore per batch row (B == 8).

Per-core plan (v6):
 - table T' in SBUF as bf16, slot g(v) = 128*(v%17) + v//17 (partition
   p = v//17, rank r = v%17). The host pads pos/new_h with zero rows to
   17*128 rows, so each staging load is ONE contiguous-per-partition DMA
   (128 descriptors of 8.5KB); host remaps indices to g(idx) and passes
   nh = concat([zeros, h]). The sink slot is zero via the padding.
 - indices/mask loaded contiguously into [16, S*N/16] int32 (partition =
   s//128, col = (s%128)*N + n). ONE DVE copy_predicated on int16 views
   does masked-select + int32->int16 + the (k w n)->(k n w) permutation
   in a single pass into a sink-prefilled idxbuf; replicate to the 8
   16-partition groups by doubling.
 - gpsimd.load_library(mlp) issued first so the Q7 ext-isa reload
   (~12us) overlaps the prologue instead of stalling the first gather.
 - SBUF->SBUF transposed dma_gather, 512 idx/call (ucode ring ceiling),
   4 SWDGE queues; desc-gen on GpSimd is the serial bottleneck
   (~2.9us/call, ~370us total).
 - pipeline at 4096-position chunk granularity (8 calls + 32 matmuls per
   chunk, 4 chunk tiles in flight): psum row m = 16w + p -> out row
   s = 128p + 8k + w via a strided DRAM AP on the output DMA.

IMPORTANT: all 2-read DVE ops (copy_predicated, tensor_add) must finish
before any dma_gather runs -- the gather ucode streams its indices
through the POOL/DVE shared SBUF read port, and a concurrent 2-port DVE
op corrupts the stream. All gathers transitively depend on the DVE
prologue via tbl/idxbuf.
"""
import sys

sys.path.insert(0, '/opt/trn_rl_repo')

import numpy as np

import concourse.bacc as bacc
import concourse.bass as bass
import concourse.mybir as mybir
import concourse.tile as tile
from concourse import library_config
from concourse.bass_utils import run_bass_kernel_spmd
from concourse.masks import make_identity

B, N, H = 8, 32, 128
NI = 512             # idxs per dma_gather call (ucode ring ceiling)
CHUNK = 4096         # gather positions per pipeline chunk (= 128 s rows)
F32 = mybir.dt.float32
I32 = mybir.dt.int32
I16 = mybir.dt.int16
BF16 = mybir.dt.bfloat16


def build_program(S: int = 2048, ni: int = NI):
    VPOS = S + 1                      # pos_table rows; sink index == VPOS
    NRANKS = (VPOS + 1 + 127) // 128  # table ranks (17 for S=2048)
    nchunk = S * N // CHUNK
    calls = CHUNK // ni               # gather calls per chunk
    assert S % 128 == 0 and CHUNK % ni == 0
    assert ni <= 512, "dma_gather ucode ring ceiling is 512 idxs/call"

    nc = bacc.Bacc("TRN2", debug=False, num_swdge_queues=4)
    # nh = reference's new_h = concat([zeros(1,H), h]) padded with zero rows
    # to NRANKS*128 rows; pos likewise zero-padded.
    nh_d = nc.dram_tensor("nh", [NRANKS * 128, H], F32, kind="ExternalInput")
    # cycling sink-slot pattern (zero table slots spread over partitions
    # 121..127 so masked reads don't hammer one partition)
    snk_d = nc.dram_tensor("snk", [16, S * N // 16], I16, kind="ExternalInput")
    idx_d = nc.dram_tensor("idx", [S, N], I32, kind="ExternalInput")
    msk_d = nc.dram_tensor("msk", [S, N], I32, kind="ExternalInput")
    pos_d = nc.dram_tensor("pos", [NRANKS * 128, H], F32, kind="ExternalInput")
    wn_d = nc.dram_tensor("wn", [H, H], F32, kind="ExternalInput")
    out_d = nc.dram_tensor("out", [S, H], F32, kind="ExternalOutput")

    with tile.TileContext(nc) as tc:
        with (
            tc.tile_pool(name="const", bufs=1) as constp,
            tc.tile_pool(name="stage", bufs=1) as stagep,
            tc.tile_pool(name="idxp", bufs=1) as idxp,
            tc.tile_pool(name="gbig", bufs=4) as gbigp,
            tc.tile_pool(name="outp", bufs=4) as outp,
            tc.tile_pool(name="psum", bufs=4, space="PSUM") as psump,
        ):
            # preload the Q7 ext-isa library holding DMAGatherAnt so the
            # ~12us IRAM reload overlaps the rest of the prologue
            nc.gpsimd.load_library(library_config.mlp)

            acols = S * N // 16
            # ---- index/mask loads (head of BOTH rings) ---------------
            idxw32 = idxp.tile([16, acols], I32, tag="idxw32")
            mskw32 = idxp.tile([16, acols], I32, tag="mskw32")
            nc.sync.dma_start(
                idxw32[:], idx_d[:].rearrange("(p r) n -> p (r n)", p=16)
            )
            nc.scalar.dma_start(
                mskw32[:], msk_d[:].rearrange("(p r) n -> p (r n)", p=16)
            )

            # ---- table staging loads ---------------------------------
            # slot g(v) = 128*(v%17) + v//17: tbl[p, r*H:(r+1)*H] =
            # T'[17p + r]; one contiguous descriptor per partition.
            pstage = stagep.tile([128, NRANKS * H], F32, tag="pstage")
            hstage = stagep.tile([128, NRANKS * H], F32, tag="hstage")
            nc.sync.dma_start(
                pstage[:], pos_d[:].rearrange("(p r) e -> p (r e)", p=128)
            )
            nc.scalar.dma_start(
                hstage[:], nh_d[:].rearrange("(p r) e -> p (r e)", p=128)
            )
            wn_sb = constp.tile([H, H], F32)
            nc.scalar.dma_start(wn_sb[:], wn_d[:])

            # ---- fused masked-select + int16 convert + permutation ---
            # idxbuf[0:16] prefilled with cycling sink slots (all zero table
            # rows; spread over partitions 121..127 -- NOT 0-31, whose AXI
            # ports serve the SWDGE descriptor rings); ONE copy_predicated
            # on int16 views then writes g(idx) where mask!=0, applying the
            # (k w n) -> (k n w) permutation: position col = 256k + 8n + w
            # reads idx[s = 128p + 8k + w, n].
            idxbuf = idxp.tile([128, acols], I16, tag="idxbuf")
            nc.sync.dma_start(idxbuf[0:16, :], snk_d[:])
            dst = idxbuf[0:16, :].rearrange(
                "p (k n w one) -> p k n w one", n=N, w=8, one=1
            )
            data = idxw32[:].bitcast(I16).rearrange(
                "p (k w n two) -> p k n w two", w=8, n=N, two=2
            )
            mask = mskw32[:].bitcast(I16).rearrange(
                "p (k w n two) -> p k n w two", w=8, n=N, two=2
            )
            nc.vector.copy_predicated(
                dst, mask[:, :, :, :, 0:1], data[:, :, :, :, 0:1]
            )
            # replicate to the 8 16-partition groups by doubling
            nc.sync.dma_start(idxbuf[16:32, :], idxbuf[0:16, :])
            nc.sync.dma_start(idxbuf[32:64, :], idxbuf[0:32, :])
            nc.sync.dma_start(idxbuf[64:128, :], idxbuf[0:64, :])

            # ---- table add (DVE) + Wn^T * (1/N) ----------------------
            tbl = constp.tile([128, NRANKS * H], BF16)
            nc.vector.tensor_add(tbl[:], pstage[:], hstage[:])
            ident = constp.tile([128, 128], F32)
            make_identity(nc, ident[:])
            wnt_ps = psump.tile([128, H], F32)
            nc.tensor.transpose(out=wnt_ps[:], in_=wn_sb[:], identity=ident[:])
            wnt = constp.tile([H, H], BF16)
            nc.scalar.mul(wnt[:], wnt_ps[:], 1.0 / N)

            for kg in range(nchunk):
                # ---- gathers: chunk kg covers positions [4096kg, ...) -
                gb = gbigp.tile([128, 1, CHUNK], BF16, tag="gb")
                for c in range(calls):
                    ci = kg * calls + c
                    nc.gpsimd.dma_gather(
                        gb[:, :, c * ni:(c + 1) * ni],
                        tbl[:],
                        idxbuf[:, ci * (ni // 16):(ci + 1) * (ni // 16)],
                        ni, ni, H,
                        transpose=True,
                        queue_num=ci % 4,
                        sbuf_tokens_per_rank=128,
                        sbuf_free_dim_per_rank=H * 2,
                    )

                # ---- matmuls: psum[m,k] += g[h, cols]^T @ wnt --------
                # column j = 128n + 16w + p holds (s = 128p + 8kg + w, n)
                gv = gb[:, 0, :]
                ps = psump.tile([128, H], F32, tag="ps")
                for n in range(N):
                    nc.tensor.matmul(
                        out=ps[:],
                        lhsT=gv[:, 128 * n:128 * n + 128],
                        rhs=wnt[:],
                        start=(n == 0),
                        stop=(n == N - 1),
                    )
                osb = outp.tile([128, H], F32, tag="osb")
                nc.vector.tensor_copy(osb[:], ps[:])
                # psum row m = 16w + p -> out row s = 128p + 8kg + w
                dst2 = out_d[:].rearrange("(p r) e -> p r e", p=16)[
                    :, 8 * kg:8 * kg + 8, :
                ].rearrange("p w e -> w p e")
                nc.sync.dma_start(dst2, osb[:])

    nc.compile()
    return nc


_CACHE: dict[tuple, object] = {}


def _get_program(S: int, ni: int = NI):
    key = (S, ni)
    if key not in _CACHE:
        _CACHE[key] = build_program(S, ni)
    return _CACHE[key]


def _prep_host(h, idx, pos, s):
    """Host-side layout prep: zero-padded new_h/pos tables, indices
    remapped to slots g(v) = 128*(v%17) + v//17, and the cycling
    sink-slot pattern (see build_program)."""
    nranks = (s + 2 + 127) // 128
    rows = nranks * 128
    nh = np.zeros((B, rows, H), dtype=np.float32)
    nh[:, 1:s + 1] = h          # row v holds new_h[v] = h[v-1]; row 0 zero
    pos_pad = np.zeros((rows, H), dtype=np.float32)
    pos_pad[:s + 1] = pos
    idx_g = (128 * (idx % nranks) + idx // nranks).astype(np.int32)
    # sink pattern: table rows 17p + r with p in [121,128) are all beyond
    # s+1 -> zero; cycle over 7 partitions x nranks ranks of them
    acols = s * N // 16
    p16, col = np.meshgrid(np.arange(16), np.arange(acols), indexing="ij")
    m = col * 16 + p16
    snk = (128 * ((m // 7) % nranks) + 121 + m % 7).astype(np.int16)
    return nh, pos_pad, idx_g, snk


def kernel(x, h, g, neighbor_index, neighbor_mask, pos_table, Wn):
    """Full inputs in, full output out. x and g are unused by the math
    (g only provides the zero row shape; x is unused in the reference)."""
    h = np.asarray(h, dtype=np.float32)
    idx = np.asarray(neighbor_index)
    msk = np.asarray(neighbor_mask)
    pos = np.asarray(pos_table, dtype=np.float32)
    wn = np.ascontiguousarray(np.asarray(Wn), dtype=np.float32)
    b, s, n = idx.shape
    assert (b, n) == (B, N) and h.shape == (B, s, H)

    nc = _get_program(s)
    nh, pos_pad, idx_g, snk = _prep_host(h, idx, pos, s)
    snk = np.ascontiguousarray(snk)
    in_maps = [
        {
            "nh": np.ascontiguousarray(nh[c]),
            "idx": np.ascontiguousarray(idx_g[c]),
            "msk": np.ascontiguousarray(msk[c], dtype=np.int32),
            "pos": pos_pad,
            "wn": wn,
            "snk": snk,
        }
        for c in range(B)
    ]
    res = run_bass_kernel_spmd(nc, in_maps, core_ids=list(range(B)))
    return np.stack([res.results[c]["out"] for c in range(B)], axis=0)


# revision 33
# speedup vs baseline: 1.9531x; 1.3028x over previous
"""Trainium2 Bass kernel for nn_Neighbor_Mean (gnn message passing).

Math: out[b,s,:] = mean_n( mask[b,s,n] * (T_b[idx[b,s,n]] @ Wn^T) )
 with T_b[v] = pos_table[v] + (h[b][v-1] if v>=1 else 0)   (v in [0, 2049))
Since the mask multiplies matmul outputs and matmul is linear:
 out[b,s,:] = ( (1/N) * sum_n T'_b[idx_eff[b,s,n]] ) @ Wn^T
 where T' has an extra zero row at SINK=S+1 and idx_eff = mask ? idx : SINK.

Sharding: data-parallel over batch, one NeuronCore per batch row (B == 8).

Per-core plan (v6):
 - table T' in SBUF as bf16, slot g(v) = 128*(v%17) + v//17 (partition
   p = v//17, rank r = v%17). The host pads pos/new_h with zero rows to
   17*128 rows, so each staging load is ONE contiguous-per-partition DMA
   (128 descriptors of 8.5KB); host remaps indices to g(idx) and passes
   nh = concat([zeros, h]). The sink slot is zero via the padding.
 - indices/mask loaded contiguously into [16, S*N/16] int32 (partition =
   s//128, col = (s%128)*N + n). ONE DVE copy_predicated on int16 views
   does masked-select + int32->int16 + the (k w n)->(k n w) permutation
   in a single pass into a sink-prefilled idxbuf; replicate to the 8
   16-partition groups by doubling.
 - gpsimd.load_library(mlp) issued first so the Q7 ext-isa reload
   (~12us) overlaps the prologue instead of stalling the first gather.
 - SBUF->SBUF transposed dma_gather, 512 idx/call (ucode ring ceiling),
   4 SWDGE queues; desc-gen on GpSimd is the serial bottleneck
   (~2.9us/call, ~370us total).
 - pipeline at 4096-position chunk granularity (8 calls + 32 matmuls per
   chunk, 4 chunk tiles in flight): psum row m = 16w + p -> out row
   s = 128p + 8k + w via a strided DRAM AP on the output DMA.

IMPORTANT: all 2-read DVE ops (copy_predicated, tensor_add) must finish
before any dma_gather runs -- the gather ucode streams its indices
through the POOL/DVE shared SBUF read port, and a concurrent 2-port DVE
op corrupts the stream. All gathers transitively depend on the DVE
prologue via tbl/idxbuf.
"""
import sys

sys.path.insert(0, '/opt/trn_rl_repo')

import numpy as np

import concourse.bacc as bacc
import concourse.bass as bass
import concourse.mybir as mybir
import concourse.tile as tile
from concourse import library_config
from concourse.bass_utils import run_bass_kernel_spmd
from concourse.masks import make_identity

B, N, H = 8, 32, 128
NI = 512             # idxs per dma_gather call (ucode ring ceiling)
CHUNK = 4096         # gather positions per pipeline chunk (= 128 s rows)
F32 = mybir.dt.float32
I32 = mybir.dt.int32
I16 = mybir.dt.int16
BF16 = mybir.dt.bfloat16


def build_program(S: int = 2048, ni: int = NI):
    VPOS = S + 1                      # pos_table rows; sink index == VPOS
    NRANKS = (VPOS + 1 + 127) // 128  # data ranks (17 for S=2048)
    TRANKS = NRANKS + 1               # +1 all-zero rank for spread sinks
    nchunk = S * N // CHUNK
    calls = CHUNK // ni               # gather calls per chunk
    assert S % 128 == 0 and CHUNK % ni == 0
    assert ni <= 512, "dma_gather ucode ring ceiling is 512 idxs/call"

    nc = bacc.Bacc("TRN2", debug=False, num_swdge_queues=4)
    # nh = reference's new_h = concat([zeros(1,H), h]) padded with zero rows
    # to NRANKS*128 rows; pos likewise zero-padded.
    nh_d = nc.dram_tensor("nh", [TRANKS * 128, H], F32, kind="ExternalInput")
    # cycling sink-slot pattern (zero table slots spread over partitions
    # 121..127 so masked reads don't hammer one partition)
    snk_d = nc.dram_tensor("snk", [16, S * N // 16], I16, kind="ExternalInput")
    idx_d = nc.dram_tensor("idx", [S, N], I32, kind="ExternalInput")
    msk_d = nc.dram_tensor("msk", [S, N], I32, kind="ExternalInput")
    pos_d = nc.dram_tensor("pos", [TRANKS * 128, H], F32, kind="ExternalInput")
    wn_d = nc.dram_tensor("wn", [H, H], F32, kind="ExternalInput")
    out_d = nc.dram_tensor("out", [S, H], F32, kind="ExternalOutput")

    with tile.TileContext(nc) as tc:
        with (
            tc.tile_pool(name="const", bufs=1) as constp,
            tc.tile_pool(name="stage", bufs=1) as stagep,
            tc.tile_pool(name="idxp", bufs=1) as idxp,
            tc.tile_pool(name="gbig", bufs=4) as gbigp,
            tc.tile_pool(name="outp", bufs=4) as outp,
            tc.tile_pool(name="psum", bufs=4, space="PSUM") as psump,
        ):
            # preload the Q7 ext-isa library holding DMAGatherAnt so the
            # ~12us IRAM reload overlaps the rest of the prologue
            nc.gpsimd.load_library(library_config.mlp)

            acols = S * N // 16
            # ---- sink-pattern + index/mask loads (ring heads) --------
            idxbuf = idxp.tile([128, acols], I16, tag="idxbuf")
            nc.sync.dma_start(idxbuf[0:16, :], snk_d[:])
            idxw32 = idxp.tile([16, acols], I32, tag="idxw32")
            mskw32 = idxp.tile([16, acols], I32, tag="mskw32")
            nc.sync.dma_start(
                idxw32[:], idx_d[:].rearrange("(p r) n -> p (r n)", p=16)
            )
            nc.scalar.dma_start(
                mskw32[:], msk_d[:].rearrange("(p r) n -> p (r n)", p=16)
            )

            # ---- table staging loads ---------------------------------
            # slot g(v) = 128*(v%17) + v//17: tbl[p, r*H:(r+1)*H] =
            # T'[17p + r]; one contiguous descriptor per partition.
            pstage = stagep.tile([128, TRANKS * H], F32, tag="pstage")
            hstage = stagep.tile([128, TRANKS * H], F32, tag="hstage")
            nc.sync.dma_start(
                pstage[:], pos_d[:].rearrange("(p r) e -> p (r e)", p=128)
            )
            nc.scalar.dma_start(
                hstage[:], nh_d[:].rearrange("(p r) e -> p (r e)", p=128)
            )
            wn_sb = constp.tile([H, H], F32)
            nc.scalar.dma_start(wn_sb[:], wn_d[:])

            # ---- fused masked-select + int16 convert + permutation ---
            # idxbuf[0:16] prefilled with cycling sink slots (all zero table
            # rows; spread over partitions 121..127 -- NOT 0-31, whose AXI
            # ports serve the SWDGE descriptor rings); ONE copy_predicated
            # on int16 views then writes g(idx) where mask!=0, applying the
            # (k w n) -> (k n w) permutation: position col = 256k + 8n + w
            # reads idx[s = 128p + 8k + w, n].
            dst = idxbuf[0:16, :].rearrange(
                "p (k n w one) -> p k n w one", n=N, w=8, one=1
            )
            data = idxw32[:].bitcast(I16).rearrange(
                "p (k w n two) -> p k n w two", w=8, n=N, two=2
            )
            mask = mskw32[:].bitcast(I16).rearrange(
                "p (k w n two) -> p k n w two", w=8, n=N, two=2
            )
            nc.vector.copy_predicated(
                dst, mask[:, :, :, :, 0:1], data[:, :, :, :, 0:1]
            )
            # replicate to the 8 16-partition groups by doubling
            nc.sync.dma_start(idxbuf[16:32, :], idxbuf[0:16, :])
            nc.sync.dma_start(idxbuf[32:64, :], idxbuf[0:32, :])
            nc.sync.dma_start(idxbuf[64:128, :], idxbuf[0:64, :])

            # ---- table add (DVE) + Wn^T * (1/N) ----------------------
            tbl = constp.tile([128, TRANKS * H], BF16)
            nc.vector.tensor_add(tbl[:], pstage[:], hstage[:])
            ident = constp.tile([128, 128], F32)
            make_identity(nc, ident[:])
            wnt_ps = psump.tile([128, H], F32)
            nc.tensor.transpose(out=wnt_ps[:], in_=wn_sb[:], identity=ident[:])
            wnt = constp.tile([H, H], BF16)
            nc.scalar.mul(wnt[:], wnt_ps[:], 1.0 / N)

            for kg in range(nchunk):
                # ---- gathers: chunk kg covers positions [4096kg, ...) -
                gb = gbigp.tile([128, 1, CHUNK], BF16, tag="gb")
                for c in range(calls):
                    ci = kg * calls + c
                    nc.gpsimd.dma_gather(
                        gb[:, :, c * ni:(c + 1) * ni],
                        tbl[:],
                        idxbuf[:, ci * (ni // 16):(ci + 1) * (ni // 16)],
                        ni, ni, H,
                        transpose=True,
                        queue_num=ci % 4,
                        sbuf_tokens_per_rank=128,
                        sbuf_free_dim_per_rank=H * 2,
                    )

                # ---- matmuls: psum[m,k] += g[h, cols]^T @ wnt --------
                # column j = 128n + 16w + p holds (s = 128p + 8kg + w, n)
                gv = gb[:, 0, :]
                ps = psump.tile([128, H], F32, tag="ps")
                for n in range(N):
                    nc.tensor.matmul(
                        out=ps[:],
                        lhsT=gv[:, 128 * n:128 * n + 128],
                        rhs=wnt[:],
                        start=(n == 0),
                        stop=(n == N - 1),
                    )
                osb = outp.tile([128, H], F32, tag="osb")
                nc.vector.tensor_copy(osb[:], ps[:])
                # psum row m = 16w + p -> out row s = 128p + 8kg + w
                dst2 = out_d[:].rearrange("(p r) e -> p r e", p=16)[
                    :, 8 * kg:8 * kg + 8, :
                ].rearrange("p w e -> w p e")
                nc.sync.dma_start(dst2, osb[:])

    nc.compile()
    return nc


_CACHE: dict[tuple, object] = {}


def _get_program(S: int, ni: int = NI):
    key = (S, ni)
    if key not in _CACHE:
        _CACHE[key] = build_program(S, ni)
    return _CACHE[key]


def _prep_host(h, idx, pos, s):
    """Host-side layout prep: new_h/pos tables zero-padded AND interleaved
    with one zero row per 17 (DRAM row 18p + r holds T'-source row
    17p + r for r<17; r=17 rows are zero), indices remapped to slots
    g(v) = 128*(v%17) + v//17, and a sink pattern cycling over the
    all-zero rank-17 slots of partitions 32..127 (see build_program)."""
    nranks = (s + 2 + 127) // 128
    # data tables in the 17-per-partition layout
    d17 = np.zeros((B + 1, nranks * 128, H), dtype=np.float32)
    d17[:B, 1:s + 1] = h        # row v holds new_h[v] = h[v-1]; row 0 zero
    d17[B, :s + 1] = pos
    # interleave a zero row after every 17: [128, 17, H] -> [128, 18, H]
    d18 = np.zeros((B + 1, 128, nranks + 1, H), dtype=np.float32)
    d18[:, :, :nranks] = d17.reshape(B + 1, 128, nranks, H)
    d18 = d18.reshape(B + 1, (nranks + 1) * 128, H)
    nh, pos_pad = d18[:B], d18[B]
    idx_g = (128 * (idx % nranks) + idx // nranks).astype(np.int32)
    # sink pattern: rank-17 slots (128*17 + p) of partitions 32..127
    acols = s * N // 16
    p16, col = np.meshgrid(np.arange(16), np.arange(acols), indexing="ij")
    m = col * 16 + p16
    snk = (128 * nranks + 32 + m % 96).astype(np.int16)
    return nh, pos_pad, idx_g, snk


def kernel(x, h, g, neighbor_index, neighbor_mask, pos_table, Wn):
    """Full inputs in, full output out. x and g are unused by the math
    (g only provides the zero row shape; x is unused in the reference)."""
    h = np.asarray(h, dtype=np.float32)
    idx = np.asarray(neighbor_index)
    msk = np.asarray(neighbor_mask)
    pos = np.asarray(pos_table, dtype=np.float32)
    wn = np.ascontiguousarray(np.asarray(Wn), dtype=np.float32)
    b, s, n = idx.shape
    assert (b, n) == (B, N) and h.shape == (B, s, H)

    nc = _get_program(s)
    nh, pos_pad, idx_g, snk = _prep_host(h, idx, pos, s)
    snk = np.ascontiguousarray(snk)
    in_maps = [
        {
            "nh": np.ascontiguousarray(nh[c]),
            "idx": np.ascontiguousarray(idx_g[c]),
            "msk": np.ascontiguousarray(msk[c], dtype=np.int32),
            "pos": pos_pad,
            "wn": wn,
            "snk": snk,
        }
        for c in range(B)
    ]
    res = run_bass_kernel_spmd(nc, in_maps, core_ids=list(range(B)))
    return np.stack([res.results[c]["out"] for c in range(B)], axis=0)


# revision 36
# speedup vs baseline: 1.9722x; 1.0098x over previous
"""Trainium2 Bass kernel for nn_Neighbor_Mean (gnn message passing).

Math: out[b,s,:] = mean_n( mask[b,s,n] * (T_b[idx[b,s,n]] @ Wn^T) )
 with T_b[v] = pos_table[v] + (h[b][v-1] if v>=1 else 0)   (v in [0, 2049))
Since the mask multiplies matmul outputs and matmul is linear:
 out[b,s,:] = ( (1/N) * sum_n T'_b[idx_eff[b,s,n]] ) @ Wn^T
 where T' has an extra zero row at SINK=S+1 and idx_eff = mask ? idx : SINK.

Sharding: data-parallel over batch, one NeuronCore per batch row (B == 8).

Per-core plan (v6):
 - table T' in SBUF as bf16, slot g(v) = 128*(v%17) + v//17 (partition
   p = v//17, rank r = v%17). The host pads pos/new_h with zero rows to
   17*128 rows, so each staging load is ONE contiguous-per-partition DMA
   (128 descriptors of 8.5KB); host remaps indices to g(idx) and passes
   nh = concat([zeros, h]). The sink slot is zero via the padding.
 - indices/mask loaded contiguously into [16, S*N/16] int32 (partition =
   s//128, col = (s%128)*N + n). ONE DVE copy_predicated on int16 views
   does masked-select + int32->int16 + the (k w n)->(k n w) permutation
   in a single pass into a sink-prefilled idxbuf; replicate to the 8
   16-partition groups by doubling.
 - gpsimd.load_library(mlp) issued first so the Q7 ext-isa reload
   (~12us) overlaps the prologue instead of stalling the first gather.
 - SBUF->SBUF transposed dma_gather, 512 idx/call (ucode ring ceiling),
   4 SWDGE queues; desc-gen on GpSimd is the serial bottleneck
   (~2.9us/call, ~370us total).
 - pipeline at 4096-position chunk granularity (8 calls + 32 matmuls per
   chunk, 4 chunk tiles in flight): psum row m = 16w + p -> out row
   s = 128p + 8k + w via a strided DRAM AP on the output DMA.

IMPORTANT: all 2-read DVE ops (copy_predicated, tensor_add) must finish
before any dma_gather runs -- the gather ucode streams its indices
through the POOL/DVE shared SBUF read port, and a concurrent 2-port DVE
op corrupts the stream. All gathers transitively depend on the DVE
prologue via tbl/idxbuf.
"""
import sys

sys.path.insert(0, '/opt/trn_rl_repo')

import numpy as np

import concourse.bacc as bacc
import concourse.bass as bass
import concourse.mybir as mybir
import concourse.tile as tile
from concourse import library_config
from concourse.bass_utils import run_bass_kernel_spmd
from concourse.masks import make_identity

B, N, H = 8, 32, 128
NI = 512             # idxs per dma_gather call (ucode ring ceiling)
CHUNK = 4096         # gather positions per pipeline chunk (= 128 s rows)
F32 = mybir.dt.float32
I32 = mybir.dt.int32
I16 = mybir.dt.int16
BF16 = mybir.dt.bfloat16


def build_program(S: int = 2048, ni: int = NI):
    VPOS = S + 1                      # pos_table rows; sink index == VPOS
    # table lives on partitions [32, 128) only -- partitions 0-31's AXI
    # ports serve the SWDGE descriptor rings and must not see table reads
    PDATA = 96                        # data partitions (32..127)
    DR = (VPOS + 1 + PDATA - 1) // PDATA  # data ranks (22 for S=2048)
    TRANKS = DR + 1                   # +1 all-zero rank for spread sinks
    nchunk = S * N // CHUNK
    calls = CHUNK // ni               # gather calls per chunk
    assert S % 128 == 0 and CHUNK % ni == 0
    assert ni <= 512, "dma_gather ucode ring ceiling is 512 idxs/call"

    nc = bacc.Bacc("TRN2", debug=False, num_swdge_queues=4)
    # nh = reference's new_h = concat([zeros(1,H), h]) padded with zero rows
    # to NRANKS*128 rows; pos likewise zero-padded.
    nh_d = nc.dram_tensor("nh", [TRANKS * PDATA, H], F32, kind="ExternalInput")
    # cycling sink-slot pattern (zero table slots spread over partitions
    # 121..127 so masked reads don't hammer one partition)
    snk_d = nc.dram_tensor("snk", [16, S * N // 16], I16, kind="ExternalInput")
    idx_d = nc.dram_tensor("idx", [S, N], I32, kind="ExternalInput")
    msk_d = nc.dram_tensor("msk", [S, N], I32, kind="ExternalInput")
    pos_d = nc.dram_tensor("pos", [TRANKS * PDATA, H], F32, kind="ExternalInput")
    wn_d = nc.dram_tensor("wn", [H, H], F32, kind="ExternalInput")
    out_d = nc.dram_tensor("out", [S, H], F32, kind="ExternalOutput")

    with tile.TileContext(nc) as tc:
        with (
            tc.tile_pool(name="const", bufs=1) as constp,
            tc.tile_pool(name="stage", bufs=1) as stagep,
            tc.tile_pool(name="idxp", bufs=1) as idxp,
            tc.tile_pool(name="gbig", bufs=4) as gbigp,
            tc.tile_pool(name="outp", bufs=4) as outp,
            tc.tile_pool(name="psum", bufs=4, space="PSUM") as psump,
        ):
            # preload the Q7 ext-isa library holding DMAGatherAnt so the
            # ~12us IRAM reload overlaps the rest of the prologue
            nc.gpsimd.load_library(library_config.mlp)

            acols = S * N // 16
            # ---- sink-pattern + index/mask loads (ring heads) --------
            idxbuf = idxp.tile([128, acols], I16, tag="idxbuf")
            nc.sync.dma_start(idxbuf[0:16, :], snk_d[:])
            idxw32 = idxp.tile([16, acols], I32, tag="idxw32")
            mskw32 = idxp.tile([16, acols], I32, tag="mskw32")
            nc.sync.dma_start(
                idxw32[:], idx_d[:].rearrange("(p r) n -> p (r n)", p=16)
            )
            nc.scalar.dma_start(
                mskw32[:], msk_d[:].rearrange("(p r) n -> p (r n)", p=16)
            )

            # ---- table staging loads ---------------------------------
            # slot g(v) = 128*(v%17) + v//17: tbl[p, r*H:(r+1)*H] =
            # T'[17p + r]; one contiguous descriptor per partition.
            pstage = stagep.tile([128, TRANKS * H], F32, tag="pstage")
            hstage = stagep.tile([128, TRANKS * H], F32, tag="hstage")
            nc.sync.dma_start(
                pstage[32:128, :],
                pos_d[:].rearrange("(p r) e -> p (r e)", p=PDATA),
            )
            nc.scalar.dma_start(
                hstage[32:128, :],
                nh_d[:].rearrange("(p r) e -> p (r e)", p=PDATA),
            )
            wn_sb = constp.tile([H, H], F32)
            nc.scalar.dma_start(wn_sb[:], wn_d[:])

            # ---- fused masked-select + int16 convert + permutation ---
            # idxbuf[0:16] prefilled with cycling sink slots (all zero table
            # rows; spread over partitions 121..127 -- NOT 0-31, whose AXI
            # ports serve the SWDGE descriptor rings); ONE copy_predicated
            # on int16 views then writes g(idx) where mask!=0, applying the
            # (k w n) -> (k n w) permutation: position col = 256k + 8n + w
            # reads idx[s = 128p + 8k + w, n].
            hcol = acols // 2
            dst = idxbuf[0:16, :].rearrange(
                "p (half k n w one) -> p half k n w one", half=2, n=N, w=8, one=1
            )
            data = idxw32[:].bitcast(I16).rearrange(
                "p (half k w n two) -> p half k n w two", half=2, w=8, n=N, two=2
            )
            mask = mskw32[:].bitcast(I16).rearrange(
                "p (half k w n two) -> p half k n w two", half=2, w=8, n=N, two=2
            )
            for hf in range(2):
                nc.vector.copy_predicated(
                    dst[:, hf], mask[:, hf, :, :, :, 0:1], data[:, hf, :, :, :, 0:1]
                )
                # replicate this half to the 8 16-partition groups
                cs = slice(hf * hcol, (hf + 1) * hcol)
                nc.sync.dma_start(idxbuf[16:32, cs], idxbuf[0:16, cs])
                nc.sync.dma_start(idxbuf[32:64, cs], idxbuf[0:32, cs])
                nc.sync.dma_start(idxbuf[64:128, cs], idxbuf[0:64, cs])

            # ---- table add (DVE) + Wn^T * (1/N) ----------------------
            tbl = constp.tile([128, TRANKS * H], BF16)
            # full-range add: partitions 0-31 hold garbage (never gathered;
            # compute ops can't start at partition 32 with >32 partitions)
            nc.vector.tensor_add(tbl[:], pstage[:], hstage[:])
            nc.vector.tensor_add(
                tbl[0:1, 0:1],
                idxbuf[0:1, hcol - 1:hcol].bitcast(BF16),
                idxbuf[0:1, acols - 1:acols].bitcast(BF16),
            )
            ident = constp.tile([128, 128], F32)
            make_identity(nc, ident[:])
            wnt_ps = psump.tile([128, H], F32)
            nc.tensor.transpose(out=wnt_ps[:], in_=wn_sb[:], identity=ident[:])
            wnt = constp.tile([H, H], BF16)
            nc.scalar.mul(wnt[:], wnt_ps[:], 1.0 / N)

            for kg in range(nchunk):
                # ---- gathers: chunk kg covers positions [4096kg, ...) -
                gb = gbigp.tile([128, 1, CHUNK], BF16, tag="gb")
                for c in range(calls):
                    ci = kg * calls + c
                    nc.gpsimd.dma_gather(
                        gb[:, :, c * ni:(c + 1) * ni],
                        tbl[:],
                        idxbuf[:, ci * (ni // 16):(ci + 1) * (ni // 16)],
                        ni, ni, H,
                        transpose=True,
                        queue_num=ci % 4,
                        sbuf_tokens_per_rank=128,
                        sbuf_free_dim_per_rank=H * 2,
                    )

                # ---- matmuls: psum[m,k] += g[h, cols]^T @ wnt --------
                # column j = 128n + 16w + p holds (s = 128p + 8kg + w, n)
                gv = gb[:, 0, :]
                ps = psump.tile([128, H], F32, tag="ps")
                for n in range(N):
                    nc.tensor.matmul(
                        out=ps[:],
                        lhsT=gv[:, 128 * n:128 * n + 128],
                        rhs=wnt[:],
                        start=(n == 0),
                        stop=(n == N - 1),
                    )
                osb = outp.tile([128, H], F32, tag="osb")
                nc.vector.tensor_copy(osb[:], ps[:])
                # psum row m = 16w + p -> out row s = 128p + 8kg + w
                dst2 = out_d[:].rearrange("(p r) e -> p r e", p=16)[
                    :, 8 * kg:8 * kg + 8, :
                ].rearrange("p w e -> w p e")
                nc.sync.dma_start(dst2, osb[:])

    nc.compile()
    return nc


_CACHE: dict[tuple, object] = {}


def _get_program(S: int, ni: int = NI):
    key = (S, ni)
    if key not in _CACHE:
        _CACHE[key] = build_program(S, ni)
    return _CACHE[key]


def _prep_host(h, idx, pos, s):
    """Host-side layout prep for the 96-partition table (partitions 32..127,
    DR=22 data ranks + 1 zero rank): DRAM row 23p + r holds T'-source row
    22p + r for r<22 (zero for r=22); indices remapped to slots
    g(v) = 128*(v%22) + 32 + v//22; sink pattern cycles the rank-22 slots
    of partitions 32..127 (see build_program)."""
    pdata = 96
    dr = (s + 2 + pdata - 1) // pdata
    # data tables in the 22-per-partition layout
    dflat = np.zeros((B + 1, pdata * dr, H), dtype=np.float32)
    dflat[:B, 1:s + 1] = h      # row v holds new_h[v] = h[v-1]; row 0 zero
    dflat[B, :s + 1] = pos
    # interleave a zero row after every dr rows: [96, dr, H] -> [96, dr+1, H]
    dpad = np.zeros((B + 1, pdata, dr + 1, H), dtype=np.float32)
    dpad[:, :, :dr] = dflat.reshape(B + 1, pdata, dr, H)
    dpad = dpad.reshape(B + 1, (dr + 1) * pdata, H)
    nh, pos_pad = dpad[:B], dpad[B]
    idx_g = (128 * (idx % dr) + 32 + idx // dr).astype(np.int32)
    # sink pattern: rank-dr slots (128*dr + 32 + p') of partitions 32..127
    acols = s * N // 16
    p16, col = np.meshgrid(np.arange(16), np.arange(acols), indexing="ij")
    m = col * 16 + p16
    snk = (128 * dr + 32 + m % pdata).astype(np.int16)
    return nh, pos_pad, idx_g, snk


def kernel(x, h, g, neighbor_index, neighbor_mask, pos_table, Wn):
    """Full inputs in, full output out. x and g are unused by the math
    (g only provides the zero row shape; x is unused in the reference)."""
    h = np.asarray(h, dtype=np.float32)
    idx = np.asarray(neighbor_index)
    msk = np.asarray(neighbor_mask)
    pos = np.asarray(pos_table, dtype=np.float32)
    wn = np.ascontiguousarray(np.asarray(Wn), dtype=np.float32)
    b, s, n = idx.shape
    assert (b, n) == (B, N) and h.shape == (B, s, H)

    nc = _get_program(s)
    nh, pos_pad, idx_g, snk = _prep_host(h, idx, pos, s)
    snk = np.ascontiguousarray(snk)
    in_maps = [
        {
            "nh": np.ascontiguousarray(nh[c]),
            "idx": np.ascontiguousarray(idx_g[c]),
            "msk": np.ascontiguousarray(msk[c], dtype=np.int32),
            "pos": pos_pad,
            "wn": wn,
            "snk": snk,
        }
        for c in range(B)
    ]
    res = run_bass_kernel_spmd(nc, in_maps, core_ids=list(range(B)))
    return np.stack([res.results[c]["out"] for c in range(B)], axis=0)


# revision 37
# speedup vs baseline: 2.1743x; 1.1025x over previous
"""Trainium2 Bass kernel for nn_Neighbor_Mean (gnn message passing).

Math: out[b,s,:] = mean_n( mask[b,s,n] * (T_b[idx[b,s,n]] @ Wn^T) )
 with T_b[v] = pos_table[v] + (h[b][v-1] if v>=1 else 0)   (v in [0, 2049))
Since the mask multiplies matmul outputs and matmul is linear:
 out[b,s,:] = ( (1/N) * sum_n T'_b[idx_eff[b,s,n]] ) @ Wn^T
 where T' has an extra zero row at SINK=S+1 and idx_eff = mask ? idx : SINK.

Sharding: data-parallel over batch, one NeuronCore per batch row (B == 8).

Per-core plan (v6):
 - table T' in SBUF as bf16, slot g(v) = 128*(v%17) + v//17 (partition
   p = v//17, rank r = v%17). The host pads pos/new_h with zero rows to
   17*128 rows, so each staging load is ONE contiguous-per-partition DMA
   (128 descriptors of 8.5KB); host remaps indices to g(idx) and passes
   nh = concat([zeros, h]). The sink slot is zero via the padding.
 - indices/mask loaded contiguously into [16, S*N/16] int32 (partition =
   s//128, col = (s%128)*N + n). ONE DVE copy_predicated on int16 views
   does masked-select + int32->int16 + the (k w n)->(k n w) permutation
   in a single pass into a sink-prefilled idxbuf; replicate to the 8
   16-partition groups by doubling.
 - gpsimd.load_library(mlp) issued first so the Q7 ext-isa reload
   (~12us) overlaps the prologue instead of stalling the first gather.
 - SBUF->SBUF transposed dma_gather, 512 idx/call (ucode ring ceiling),
   4 SWDGE queues; desc-gen on GpSimd is the serial bottleneck
   (~2.9us/call, ~370us total).
 - pipeline at 4096-position chunk granularity (8 calls + 32 matmuls per
   chunk, 4 chunk tiles in flight): psum row m = 16w + p -> out row
   s = 128p + 8k + w via a strided DRAM AP on the output DMA.

IMPORTANT: all 2-read DVE ops (copy_predicated, tensor_add) must finish
before any dma_gather runs -- the gather ucode streams its indices
through the POOL/DVE shared SBUF read port, and a concurrent 2-port DVE
op corrupts the stream. All gathers transitively depend on the DVE
prologue via tbl/idxbuf.
"""
import sys

sys.path.insert(0, '/opt/trn_rl_repo')

import numpy as np

import concourse.bacc as bacc
import concourse.bass as bass
import concourse.mybir as mybir
import concourse.tile as tile
from concourse import library_config
from concourse.bass_utils import run_bass_kernel_spmd
from concourse.masks import make_identity

B, N, H = 8, 32, 128
NI = 512             # idxs per dma_gather call (ucode ring ceiling)
CHUNK = 4096         # gather positions per pipeline chunk (= 128 s rows)
F32 = mybir.dt.float32
I32 = mybir.dt.int32
I16 = mybir.dt.int16
BF16 = mybir.dt.bfloat16


def build_program(S: int = 2048, ni: int = NI):
    VPOS = S + 1                      # pos_table rows; sink index == VPOS
    # table lives on partitions [32, 128) only -- partitions 0-31's AXI
    # ports serve the SWDGE descriptor rings and must not see table reads
    PDATA = 96                        # data partitions (32..127)
    DR = (VPOS + 1 + PDATA - 1) // PDATA  # data ranks (22 for S=2048)
    TRANKS = DR + 1                   # +1 all-zero rank for spread sinks
    nchunk = S * N // CHUNK
    calls = CHUNK // ni               # gather calls per chunk
    assert S % 128 == 0 and CHUNK % ni == 0
    assert ni <= 512, "dma_gather ucode ring ceiling is 512 idxs/call"

    nc = bacc.Bacc("TRN2", debug=False, num_swdge_queues=4)
    # nh = reference's new_h = concat([zeros(1,H), h]) padded with zero rows
    # to NRANKS*128 rows; pos likewise zero-padded.
    nh_d = nc.dram_tensor("nh", [TRANKS * PDATA, H], F32, kind="ExternalInput")
    # cycling sink-slot pattern (zero table slots spread over partitions
    # 121..127 so masked reads don't hammer one partition)
    snk_d = nc.dram_tensor("snk", [16, S * N // 16], I16, kind="ExternalInput")
    idx_d = nc.dram_tensor("idx", [S, N], I32, kind="ExternalInput")
    msk_d = nc.dram_tensor("msk", [S, N], I32, kind="ExternalInput")
    pos_d = nc.dram_tensor("pos", [TRANKS * PDATA, H], F32, kind="ExternalInput")
    wn_d = nc.dram_tensor("wn", [H, H], F32, kind="ExternalInput")
    out_d = nc.dram_tensor("out", [S, H], F32, kind="ExternalOutput")

    with tile.TileContext(nc) as tc:
        with (
            tc.tile_pool(name="const", bufs=1) as constp,
            tc.tile_pool(name="stage", bufs=1) as stagep,
            tc.tile_pool(name="idxp", bufs=1) as idxp,
            tc.tile_pool(name="gbig", bufs=4) as gbigp,
            tc.tile_pool(name="outp", bufs=4) as outp,
            tc.tile_pool(name="psum", bufs=4, space="PSUM") as psump,
        ):
            # preload the Q7 ext-isa library holding DMAGatherAnt so the
            # ~12us IRAM reload overlaps the rest of the prologue
            nc.gpsimd.load_library(library_config.mlp)

            acols = S * N // 16
            # ---- sink-pattern + index/mask loads (ring heads) --------
            idxbuf = idxp.tile([128, acols], I16, tag="idxbuf")
            nc.sync.dma_start(idxbuf[0:16, :], snk_d[:])
            idxw32 = idxp.tile([16, acols], I32, tag="idxw32")
            mskw32 = idxp.tile([16, acols], I32, tag="mskw32")
            nc.sync.dma_start(
                idxw32[:], idx_d[:].rearrange("(p r) n -> p (r n)", p=16)
            )
            nc.scalar.dma_start(
                mskw32[:], msk_d[:].rearrange("(p r) n -> p (r n)", p=16)
            )

            # ---- table staging loads ---------------------------------
            # slot g(v) = 128*(v%17) + v//17: tbl[p, r*H:(r+1)*H] =
            # T'[17p + r]; one contiguous descriptor per partition.
            pstage = stagep.tile([128, TRANKS * H], F32, tag="pstage")
            hstage = stagep.tile([128, TRANKS * H], F32, tag="hstage")
            nc.sync.dma_start(
                pstage[32:128, :],
                pos_d[:].rearrange("(p r) e -> p (r e)", p=PDATA),
            )
            nc.scalar.dma_start(
                hstage[32:128, :],
                nh_d[:].rearrange("(p r) e -> p (r e)", p=PDATA),
            )
            wn_sb = constp.tile([H, H], F32)
            nc.scalar.dma_start(wn_sb[:], wn_d[:])

            # ---- fused masked-select + int16 convert + permutation ---
            # idxbuf[0:16] prefilled with cycling sink slots (all zero table
            # rows; spread over partitions 121..127 -- NOT 0-31, whose AXI
            # ports serve the SWDGE descriptor rings); ONE copy_predicated
            # on int16 views then writes g(idx) where mask!=0, applying the
            # (k w n) -> (k n w) permutation: position col = 256k + 8n + w
            # reads idx[s = 128p + 8k + w, n].
            hcol = acols // 2
            dst = idxbuf[0:16, :].rearrange(
                "p (half k n w one) -> p half k n w one", half=2, n=N, w=8, one=1
            )
            data = idxw32[:].bitcast(I16).rearrange(
                "p (half k w n two) -> p half k n w two", half=2, w=8, n=N, two=2
            )
            mask = mskw32[:].bitcast(I16).rearrange(
                "p (half k w n two) -> p half k n w two", half=2, w=8, n=N, two=2
            )
            for hf in range(2):
                nc.vector.copy_predicated(
                    dst[:, hf], mask[:, hf, :, :, :, 0:1], data[:, hf, :, :, :, 0:1]
                )
                # replicate this half to the 8 16-partition groups
                cs = slice(hf * hcol, (hf + 1) * hcol)
                nc.sync.dma_start(idxbuf[16:32, cs], idxbuf[0:16, cs])
                nc.sync.dma_start(idxbuf[32:64, cs], idxbuf[0:32, cs])
                nc.sync.dma_start(idxbuf[64:128, cs], idxbuf[0:64, cs])

            # ---- table add (DVE) + Wn^T * (1/N) ----------------------
            tbl = constp.tile([128, TRANKS * H], BF16)
            # full-range add on gpsimd (idle after the library reload;
            # partitions 0-31 hold garbage -- never gathered, and compute
            # ops can't start at partition 32 with >32 partitions)
            nc.gpsimd.tensor_add(tbl[:], pstage[:], hstage[:])
            # guard (DVE): reads both select halves, writes an unused tbl
            # slot -- orders every gather after all 2-port DVE prologue ops
            # AND after the main add (overlapping tbl write)
            nc.vector.tensor_add(
                tbl[0:1, 0:1],
                idxbuf[0:1, hcol - 1:hcol].bitcast(BF16),
                idxbuf[0:1, acols - 1:acols].bitcast(BF16),
            )
            ident = constp.tile([128, 128], F32)
            make_identity(nc, ident[:])
            wnt_ps = psump.tile([128, H], F32)
            nc.tensor.transpose(out=wnt_ps[:], in_=wn_sb[:], identity=ident[:])
            wnt = constp.tile([H, H], BF16)
            nc.scalar.mul(wnt[:], wnt_ps[:], 1.0 / N)

            for kg in range(nchunk):
                # ---- gathers: chunk kg covers positions [4096kg, ...) -
                gb = gbigp.tile([128, 1, CHUNK], BF16, tag="gb")
                for c in range(calls):
                    ci = kg * calls + c
                    nc.gpsimd.dma_gather(
                        gb[:, :, c * ni:(c + 1) * ni],
                        tbl[:],
                        idxbuf[:, ci * (ni // 16):(ci + 1) * (ni // 16)],
                        ni, ni, H,
                        transpose=True,
                        queue_num=ci % 4,
                        sbuf_tokens_per_rank=128,
                        sbuf_free_dim_per_rank=H * 2,
                    )

                # ---- matmuls: psum[m,k] += g[h, cols]^T @ wnt --------
                # column j = 128n + 16w + p holds (s = 128p + 8kg + w, n)
                gv = gb[:, 0, :]
                ps = psump.tile([128, H], F32, tag="ps")
                for n in range(N):
                    nc.tensor.matmul(
                        out=ps[:],
                        lhsT=gv[:, 128 * n:128 * n + 128],
                        rhs=wnt[:],
                        start=(n == 0),
                        stop=(n == N - 1),
                    )
                osb = outp.tile([128, H], F32, tag="osb")
                nc.vector.tensor_copy(osb[:], ps[:])
                # psum row m = 16w + p -> out row s = 128p + 8kg + w
                dst2 = out_d[:].rearrange("(p r) e -> p r e", p=16)[
                    :, 8 * kg:8 * kg + 8, :
                ].rearrange("p w e -> w p e")
                nc.sync.dma_start(dst2, osb[:])

    nc.compile()
    return nc


_CACHE: dict[tuple, object] = {}


def _get_program(S: int, ni: int = NI):
    key = (S, ni)
    if key not in _CACHE:
        _CACHE[key] = build_program(S, ni)
    return _CACHE[key]


def _prep_host(h, idx, pos, s):
    """Host-side layout prep for the 96-partition table (partitions 32..127,
    DR=22 data ranks + 1 zero rank): DRAM row 23p + r holds T'-source row
    22p + r for r<22 (zero for r=22); indices remapped to slots
    g(v) = 128*(v%22) + 32 + v//22; sink pattern cycles the rank-22 slots
    of partitions 32..127 (see build_program)."""
    pdata = 96
    dr = (s + 2 + pdata - 1) // pdata
    # data tables in the 22-per-partition layout
    dflat = np.zeros((B + 1, pdata * dr, H), dtype=np.float32)
    dflat[:B, 1:s + 1] = h      # row v holds new_h[v] = h[v-1]; row 0 zero
    dflat[B, :s + 1] = pos
    # interleave a zero row after every dr rows: [96, dr, H] -> [96, dr+1, H]
    dpad = np.zeros((B + 1, pdata, dr + 1, H), dtype=np.float32)
    dpad[:, :, :dr] = dflat.reshape(B + 1, pdata, dr, H)
    dpad = dpad.reshape(B + 1, (dr + 1) * pdata, H)
    nh, pos_pad = dpad[:B], dpad[B]
    idx_g = (128 * (idx % dr) + 32 + idx // dr).astype(np.int32)
    # sink pattern: rank-dr slots (128*dr + 32 + p') of partitions 32..127
    acols = s * N // 16
    p16, col = np.meshgrid(np.arange(16), np.arange(acols), indexing="ij")
    m = col * 16 + p16
    snk = (128 * dr + 32 + m % pdata).astype(np.int16)
    return nh, pos_pad, idx_g, snk


def kernel(x, h, g, neighbor_index, neighbor_mask, pos_table, Wn):
    """Full inputs in, full output out. x and g are unused by the math
    (g only provides the zero row shape; x is unused in the reference)."""
    h = np.asarray(h, dtype=np.float32)
    idx = np.asarray(neighbor_index)
    msk = np.asarray(neighbor_mask)
    pos = np.asarray(pos_table, dtype=np.float32)
    wn = np.ascontiguousarray(np.asarray(Wn), dtype=np.float32)
    b, s, n = idx.shape
    assert (b, n) == (B, N) and h.shape == (B, s, H)

    nc = _get_program(s)
    nh, pos_pad, idx_g, snk = _prep_host(h, idx, pos, s)
    snk = np.ascontiguousarray(snk)
    in_maps = [
        {
            "nh": np.ascontiguousarray(nh[c]),
            "idx": np.ascontiguousarray(idx_g[c]),
            "msk": np.ascontiguousarray(msk[c], dtype=np.int32),
            "pos": pos_pad,
            "wn": wn,
            "snk": snk,
        }
        for c in range(B)
    ]
    res = run_bass_kernel_spmd(nc, in_maps, core_ids=list(range(B)))
    return np.stack([res.results[c]["out"] for c in range(B)], axis=0)


# revision 38
# speedup vs baseline: 2.3361x; 1.0744x over previous
"""Trainium2 Bass kernel for nn_Neighbor_Mean (gnn message passing).

Math: out[b,s,:] = mean_n( mask[b,s,n] * (T_b[idx[b,s,n]] @ Wn^T) )
 with T_b[v] = pos_table[v] + (h[b][v-1] if v>=1 else 0)   (v in [0, 2049))
Since the mask multiplies matmul outputs and matmul is linear:
 out[b,s,:] = ( (1/N) * sum_n T'_b[idx_eff[b,s,n]] ) @ Wn^T
 where T' has an extra zero row at SINK=S+1 and idx_eff = mask ? idx : SINK.

Sharding: data-parallel over batch, one NeuronCore per batch row (B == 8).

Per-core plan (v6):
 - table T' in SBUF as bf16, slot g(v) = 128*(v%17) + v//17 (partition
   p = v//17, rank r = v%17). The host pads pos/new_h with zero rows to
   17*128 rows, so each staging load is ONE contiguous-per-partition DMA
   (128 descriptors of 8.5KB); host remaps indices to g(idx) and passes
   nh = concat([zeros, h]). The sink slot is zero via the padding.
 - indices/mask loaded contiguously into [16, S*N/16] int32 (partition =
   s//128, col = (s%128)*N + n). ONE DVE copy_predicated on int16 views
   does masked-select + int32->int16 + the (k w n)->(k n w) permutation
   in a single pass into a sink-prefilled idxbuf; replicate to the 8
   16-partition groups by doubling.
 - gpsimd.load_library(mlp) issued first so the Q7 ext-isa reload
   (~12us) overlaps the prologue instead of stalling the first gather.
 - SBUF->SBUF transposed dma_gather, 512 idx/call (ucode ring ceiling),
   4 SWDGE queues; desc-gen on GpSimd is the serial bottleneck
   (~2.9us/call, ~370us total).
 - pipeline at 4096-position chunk granularity (8 calls + 32 matmuls per
   chunk, 4 chunk tiles in flight): psum row m = 16w + p -> out row
   s = 128p + 8k + w via a strided DRAM AP on the output DMA.

IMPORTANT: all 2-read DVE ops (copy_predicated, tensor_add) must finish
before any dma_gather runs -- the gather ucode streams its indices
through the POOL/DVE shared SBUF read port, and a concurrent 2-port DVE
op corrupts the stream. All gathers transitively depend on the DVE
prologue via tbl/idxbuf.
"""
import sys

sys.path.insert(0, '/opt/trn_rl_repo')

import numpy as np

import concourse.bacc as bacc
import concourse.bass as bass
import concourse.mybir as mybir
import concourse.tile as tile
from concourse import library_config
from concourse.bass_utils import run_bass_kernel_spmd
from concourse.masks import make_identity

B, N, H = 8, 32, 128
NI = 512             # idxs per dma_gather call (ucode ring ceiling)
CHUNK = 4096         # gather positions per pipeline chunk (= 128 s rows)
F32 = mybir.dt.float32
I32 = mybir.dt.int32
I16 = mybir.dt.int16
BF16 = mybir.dt.bfloat16


def build_program(S: int = 2048, ni: int = NI):
    VPOS = S + 1                      # pos_table rows; sink index == VPOS
    # table lives on partitions [32, 128) only -- partitions 0-31's AXI
    # ports serve the SWDGE descriptor rings and must not see table reads
    PDATA = 96                        # data partitions (32..127)
    DR = (VPOS + 1 + PDATA - 1) // PDATA  # data ranks (22 for S=2048)
    TRANKS = DR + 1                   # +1 all-zero rank for spread sinks
    nchunk = S * N // CHUNK
    calls = CHUNK // ni               # gather calls per chunk
    assert S % 128 == 0 and CHUNK % ni == 0
    assert ni <= 512, "dma_gather ucode ring ceiling is 512 idxs/call"

    nc = bacc.Bacc("TRN2", debug=False, num_swdge_queues=4)
    # nh = reference's new_h = concat([zeros(1,H), h]) padded with zero rows
    # to NRANKS*128 rows; pos likewise zero-padded.
    nh_d = nc.dram_tensor("nh", [TRANKS * PDATA, H], F32, kind="ExternalInput")
    # cycling sink-slot pattern (zero table slots spread over partitions
    # 121..127 so masked reads don't hammer one partition)
    snk_d = nc.dram_tensor("snk", [16, S * N // 16], I16, kind="ExternalInput")
    idx_d = nc.dram_tensor("idx", [S, N], I32, kind="ExternalInput")
    msk_d = nc.dram_tensor("msk", [S, N], I32, kind="ExternalInput")
    pos_d = nc.dram_tensor("pos", [TRANKS * PDATA, H], F32, kind="ExternalInput")
    wn_d = nc.dram_tensor("wn", [H, H], F32, kind="ExternalInput")
    out_d = nc.dram_tensor("out", [S, H], F32, kind="ExternalOutput")

    with tile.TileContext(nc) as tc:
        with (
            tc.tile_pool(name="const", bufs=1) as constp,
            tc.tile_pool(name="stage", bufs=1) as stagep,
            tc.tile_pool(name="idxp", bufs=1) as idxp,
            tc.tile_pool(name="gbig", bufs=4) as gbigp,
            tc.tile_pool(name="outp", bufs=4) as outp,
            tc.tile_pool(name="psum", bufs=4, space="PSUM") as psump,
        ):
            # preload the Q7 ext-isa library holding DMAGatherAnt so the
            # ~12us IRAM reload overlaps the rest of the prologue
            nc.gpsimd.load_library(library_config.mlp)

            acols = S * N // 16
            # ---- sink-pattern + index/mask loads (ring heads) --------
            idxbuf = idxp.tile([128, acols], I16, tag="idxbuf")
            nc.sync.dma_start(idxbuf[0:16, :], snk_d[:])
            idxw32 = idxp.tile([16, acols], I32, tag="idxw32")
            mskw32 = idxp.tile([16, acols], I32, tag="mskw32")
            nc.sync.dma_start(
                idxw32[:], idx_d[:].rearrange("(p r) n -> p (r n)", p=16)
            )
            nc.scalar.dma_start(
                mskw32[:], msk_d[:].rearrange("(p r) n -> p (r n)", p=16)
            )

            # ---- table staging loads ---------------------------------
            # slot g(v) = 128*(v%17) + v//17: tbl[p, r*H:(r+1)*H] =
            # T'[17p + r]; one contiguous descriptor per partition.
            pstage = stagep.tile([128, TRANKS * H], F32, tag="pstage")
            hstage = stagep.tile([128, TRANKS * H], F32, tag="hstage")
            nc.sync.dma_start(
                pstage[32:128, :],
                pos_d[:].rearrange("(p r) e -> p (r e)", p=PDATA),
            )
            nc.scalar.dma_start(
                hstage[32:128, :],
                nh_d[:].rearrange("(p r) e -> p (r e)", p=PDATA),
            )
            wn_sb = constp.tile([H, H], F32)
            nc.scalar.dma_start(wn_sb[:], wn_d[:])

            # ---- fused masked-select + int16 convert + permutation ---
            # idxbuf[0:16] prefilled with cycling sink slots (all zero table
            # rows; spread over partitions 121..127 -- NOT 0-31, whose AXI
            # ports serve the SWDGE descriptor rings); ONE copy_predicated
            # on int16 views then writes g(idx) where mask!=0, applying the
            # (k w n) -> (k n w) permutation: position col = 256k + 8n + w
            # reads idx[s = 128p + 8k + w, n].
            hcol = acols // 2
            dst = idxbuf[0:16, :].rearrange(
                "p (half k n w one) -> p half k n w one", half=2, n=N, w=8, one=1
            )
            data = idxw32[:].bitcast(I16).rearrange(
                "p (half k w n two) -> p half k n w two", half=2, w=8, n=N, two=2
            )
            mask = mskw32[:].bitcast(I16).rearrange(
                "p (half k w n two) -> p half k n w two", half=2, w=8, n=N, two=2
            )
            for hf in range(2):
                nc.vector.copy_predicated(
                    dst[:, hf], mask[:, hf, :, :, :, 0:1], data[:, hf, :, :, :, 0:1]
                )
                # replicate this half to the 8 16-partition groups
                cs = slice(hf * hcol, (hf + 1) * hcol)
                nc.sync.dma_start(idxbuf[16:32, cs], idxbuf[0:16, cs])
                nc.sync.dma_start(idxbuf[32:64, cs], idxbuf[0:32, cs])
                nc.sync.dma_start(idxbuf[64:128, cs], idxbuf[0:64, cs])

            # ---- table add (DVE) + Wn^T * (1/N) ----------------------
            tbl = constp.tile([128, TRANKS * H], BF16)
            # full-range add on DVE (a gpsimd add would thrash the Q7
            # ext-isa library: TensorTensor lives in `standard`, forcing
            # reloads around the gather ucode). Partitions 0-31 hold
            # garbage -- never gathered, and compute ops can't start at
            # partition 32 with >32 partitions.
            nc.vector.tensor_add(tbl[:], pstage[:], hstage[:])
            # guard (DVE): reads both select halves, writes an unused tbl
            # slot -- orders every gather after all 2-port DVE prologue ops
            # AND after the main add (overlapping tbl write)
            nc.vector.tensor_add(
                tbl[0:1, 0:1],
                idxbuf[0:1, hcol - 1:hcol].bitcast(BF16),
                idxbuf[0:1, acols - 1:acols].bitcast(BF16),
            )
            ident = constp.tile([128, 128], F32)
            make_identity(nc, ident[:])
            wnt_ps = psump.tile([128, H], F32)
            nc.tensor.transpose(out=wnt_ps[:], in_=wn_sb[:], identity=ident[:])
            wnt = constp.tile([H, H], BF16)
            nc.scalar.mul(wnt[:], wnt_ps[:], 1.0 / N)

            for kg in range(nchunk):
                # ---- gathers: chunk kg covers positions [4096kg, ...) -
                gb = gbigp.tile([128, 1, CHUNK], BF16, tag="gb")
                for c in range(calls):
                    ci = kg * calls + c
                    nc.gpsimd.dma_gather(
                        gb[:, :, c * ni:(c + 1) * ni],
                        tbl[:],
                        idxbuf[:, ci * (ni // 16):(ci + 1) * (ni // 16)],
                        ni, ni, H,
                        transpose=True,
                        queue_num=ci % 4,
                        sbuf_tokens_per_rank=128,
                        sbuf_free_dim_per_rank=H * 2,
                    )

                # ---- matmuls: psum[m,k] += g[h, cols]^T @ wnt --------
                # column j = 128n + 16w + p holds (s = 128p + 8kg + w, n)
                gv = gb[:, 0, :]
                ps = psump.tile([128, H], F32, tag="ps")
                for n in range(N):
                    nc.tensor.matmul(
                        out=ps[:],
                        lhsT=gv[:, 128 * n:128 * n + 128],
                        rhs=wnt[:],
                        start=(n == 0),
                        stop=(n == N - 1),
                    )
                osb = outp.tile([128, H], F32, tag="osb")
                nc.vector.tensor_copy(osb[:], ps[:])
                # psum row m = 16w + p -> out row s = 128p + 8kg + w
                dst2 = out_d[:].rearrange("(p r) e -> p r e", p=16)[
                    :, 8 * kg:8 * kg + 8, :
                ].rearrange("p w e -> w p e")
                nc.sync.dma_start(dst2, osb[:])

    nc.compile()
    return nc


_CACHE: dict[tuple, object] = {}


def _get_program(S: int, ni: int = NI):
    key = (S, ni)
    if key not in _CACHE:
        _CACHE[key] = build_program(S, ni)
    return _CACHE[key]


def _prep_host(h, idx, pos, s):
    """Host-side layout prep for the 96-partition table (partitions 32..127,
    DR=22 data ranks + 1 zero rank): DRAM row 23p + r holds T'-source row
    22p + r for r<22 (zero for r=22); indices remapped to slots
    g(v) = 128*(v%22) + 32 + v//22; sink pattern cycles the rank-22 slots
    of partitions 32..127 (see build_program)."""
    pdata = 96
    dr = (s + 2 + pdata - 1) // pdata
    # data tables in the 22-per-partition layout
    dflat = np.zeros((B + 1, pdata * dr, H), dtype=np.float32)
    dflat[:B, 1:s + 1] = h      # row v holds new_h[v] = h[v-1]; row 0 zero
    dflat[B, :s + 1] = pos
    # interleave a zero row after every dr rows: [96, dr, H] -> [96, dr+1, H]
    dpad = np.zeros((B + 1, pdata, dr + 1, H), dtype=np.float32)
    dpad[:, :, :dr] = dflat.reshape(B + 1, pdata, dr, H)
    dpad = dpad.reshape(B + 1, (dr + 1) * pdata, H)
    nh, pos_pad = dpad[:B], dpad[B]
    idx_g = (128 * (idx % dr) + 32 + idx // dr).astype(np.int32)
    # sink pattern: rank-dr slots (128*dr + 32 + p') of partitions 32..127
    acols = s * N // 16
    p16, col = np.meshgrid(np.arange(16), np.arange(acols), indexing="ij")
    m = col * 16 + p16
    snk = (128 * dr + 32 + m % pdata).astype(np.int16)
    return nh, pos_pad, idx_g, snk


def kernel(x, h, g, neighbor_index, neighbor_mask, pos_table, Wn):
    """Full inputs in, full output out. x and g are unused by the math
    (g only provides the zero row shape; x is unused in the reference)."""
    h = np.asarray(h, dtype=np.float32)
    idx = np.asarray(neighbor_index)
    msk = np.asarray(neighbor_mask)
    pos = np.asarray(pos_table, dtype=np.float32)
    wn = np.ascontiguousarray(np.asarray(Wn), dtype=np.float32)
    b, s, n = idx.shape
    assert (b, n) == (B, N) and h.shape == (B, s, H)

    nc = _get_program(s)
    nh, pos_pad, idx_g, snk = _prep_host(h, idx, pos, s)
    snk = np.ascontiguousarray(snk)
    in_maps = [
        {
            "nh": np.ascontiguousarray(nh[c]),
            "idx": np.ascontiguousarray(idx_g[c]),
            "msk": np.ascontiguousarray(msk[c], dtype=np.int32),
            "pos": pos_pad,
            "wn": wn,
            "snk": snk,
        }
        for c in range(B)
    ]
    res = run_bass_kernel_spmd(nc, in_maps, core_ids=list(range(B)))
    return np.stack([res.results[c]["out"] for c in range(B)], axis=0)
